# revision 9
# baseline (speedup 1.0000x reference)
"""Trainium2 Bass kernel for nn_DotPred (gnn_message_passing).

score[t, e] = sum_d (x[src] - x[dst]) / sqrt(D)
            = s[src] - s[dst],   s = rowsum(x) / sqrt(D)

Strategy (8 NeuronCores, SPMD):
- Phase 1: rowsum sharded 8 ways (each core reduces 12800 nodes, 6.5MB),
  scaled by 1/sqrt(D), then AllGather (DRAM collective) rebuilds the full
  s table S[128, 800] on every core (node n at partition n & 127,
  column n >> 7).
- Phase 2: per-edge gather of s[src], s[dst] via one-hot matmuls.
  Host pre-sorts each core's edges by (src_block, dst_block) pair
  (block = 4096 nodes = 128 partitions x 32 columns) into 625 groups padded
  to 128-edge tiles (a core-uniform static schedule). Per 128-edge tile:
    PE poly-mm (k=8, bf16):   Q3[p, e] = 1 - (a-a_e)^2 - 2(b-b_e)^2
                              (p = 8a + b; 1 iff p == p_e, else <= 0;
                               all bf16 products are integers <= 256, exact)
    DVE/ACT relu:             OHP[p, e] = relu(Q3) in {0, 1}, bf16
    PE select-mm (fp16):      RT[e, c] = sum_p OHP[p, e] * S_f16[p, c]
    DVE:                      val[e] = sum_c RT[e, c] * mask  (bf16 mask)
  S is selected as fp16 (~2^-11 relative error, far under tolerance).
  Relus are split ACT(5)/Pool(3) per batch; DVE keeps mask-mult+reduce.
- Final: val_src - val_dst on device; host un-permutes.
"""
import math
from contextlib import ExitStack

import numpy as np
import ml_dtypes

import concourse.bass as bass
import concourse.mybir as mybir
from concourse.bass_utils import run_bass_kernel_spmd

P = 128
D = 128
CB = 32             # columns per block
NBLK = 25           # node blocks (4096 nodes each) covering 100096 nodes
N_NODES = 100000
VPAD = 102400       # 8 * 12800 (also 25 * 4096)
NCORES = 8
NSH = VPAD // NCORES    # nodes per core shard (12800)
SCOL = NSH // P         # S columns per core shard (100)
TPB = 16            # tiles per phase-2 batch (one PSUM bank of RT)
CHT = 4             # tiles per poly/relu chunk (one PSUM bank)
NCH = TPB // CHT    # chunks per batch (4)
INV_SQ = 1.0 / math.sqrt(128.0)

F32 = mybir.dt.float32
BF16 = mybir.dt.bfloat16
ALU = mybir.AluOpType
ACTF = mybir.ActivationFunctionType
NPBF = np.dtype(ml_dtypes.bfloat16)
NPF16 = np.dtype(np.float16)

F16 = mybir.dt.float16

# relu chunk ownership: (side, chunk) -> engine 'a' (ACT) / 'p' (Pool) / 'v' (DVE)
RELU_OWNER = {
    (0, 0): "v", (0, 1): "v", (0, 2): "a", (0, 3): "a",
    (1, 0): "a", (1, 1): "a", (1, 2): "a", (1, 3): "a",
}


def _build_nc(n_tiles, sched):
    assert len(sched) == n_tiles and n_tiles % TPB == 0
    nbatch = n_tiles // TPB
    CW = CHT * P         # chunk width in edges (512)
    ECH = 4              # phase-1 chunks
    EJ = SCOL // ECH     # j-columns per phase-1 chunk (25)

    nc = bass.Bass(num_devices=NCORES)
    embeds = nc.declare_dram_parameter("embeds", [NSH, D], F32, isOutput=False)
    p3_both = nc.declare_dram_parameter("p3_both", [nbatch, 16, TPB * P], F16, isOutput=False)
    mask_in = nc.declare_dram_parameter("mask_in", [nbatch, P, 2 * TPB * CB], BF16, isOutput=False)
    lhsT8_in = nc.declare_dram_parameter("lhsT8", [8, P], F16, isOutput=False)
    y = nc.declare_dram_parameter("y", [P, n_tiles], F32, isOutput=True)

    cc_in = nc.dram_tensor("cc_in", [P, SCOL], F32)
    cc_out = nc.dram_tensor("cc_out", [NCORES * P, SCOL], F32)

    es = ExitStack()
    with es:
        emb0 = es.enter_context(nc.sbuf_tensor([P, EJ * D], F32))
        emb1 = es.enter_context(nc.sbuf_tensor([P, EJ * D], F32))
        s_part = es.enter_context(nc.sbuf_tensor([P, SCOL], F32))
        S = es.enter_context(nc.sbuf_tensor([P, NBLK * CB], F32))
        S_f16 = es.enter_context(nc.sbuf_tensor([P, NBLK * CB], F16))
        lhsT8 = es.enter_context(nc.sbuf_tensor([8, P], F16))
        pS0 = es.enter_context(nc.sbuf_tensor([8, TPB * P], F16))
        pS1 = es.enter_context(nc.sbuf_tensor([8, TPB * P], F16))
        pD0 = es.enter_context(nc.sbuf_tensor([8, TPB * P], F16))
        pD1 = es.enter_context(nc.sbuf_tensor([8, TPB * P], F16))
        ohpS0 = es.enter_context(nc.sbuf_tensor([P, TPB * P], F16))
        ohpS1 = es.enter_context(nc.sbuf_tensor([P, TPB * P], F16))
        ohpD0 = es.enter_context(nc.sbuf_tensor([P, TPB * P], F16))
        ohpD1 = es.enter_context(nc.sbuf_tensor([P, TPB * P], F16))
        mb0 = es.enter_context(nc.sbuf_tensor([P, 2 * TPB * CB], BF16))
        mb1 = es.enter_context(nc.sbuf_tensor([P, 2 * TPB * CB], BF16))
        dS = es.enter_context(nc.sbuf_tensor([P, TPB * CB], F32))
        dD = es.enter_context(nc.sbuf_tensor([P, TPB * CB], F32))
        valS = es.enter_context(nc.sbuf_tensor([P, n_tiles], F32))
        valD = es.enter_context(nc.sbuf_tensor([P, n_tiles], F32))
        out_sb = es.enter_context(nc.sbuf_tensor([P, n_tiles], F32))
        qS0 = es.enter_context(nc.psum_tensor([P, CW], F32))
        qS1 = es.enter_context(nc.psum_tensor([P, CW], F32))
        qD0 = es.enter_context(nc.psum_tensor([P, CW], F32))
        qD1 = es.enter_context(nc.psum_tensor([P, CW], F32))
        psA0 = es.enter_context(nc.psum_tensor([P, TPB * CB], F32))
        psA1 = es.enter_context(nc.psum_tensor([P, TPB * CB], F32))
        psB0 = es.enter_context(nc.psum_tensor([P, TPB * CB], F32))
        psB1 = es.enter_context(nc.psum_tensor([P, TPB * CB], F32))
        ph1_load = es.enter_context(nc.semaphore())
        ph1_red = es.enter_context(nc.semaphore())
        cc_staged = es.enter_context(nc.semaphore())
        cc_done = es.enter_context(nc.semaphore())
        s_loaded = es.enter_context(nc.semaphore())
        tbl_ready = es.enter_context(nc.semaphore())
        pre_load = es.enter_context(nc.semaphore())
        ploadA = es.enter_context(nc.semaphore())
        ploadB = es.enter_context(nc.semaphore())
        mloadA = es.enter_context(nc.semaphore())
        mloadB = es.enter_context(nc.semaphore())
        ydone = es.enter_context(nc.semaphore())
        ps_done = es.enter_context(nc.semaphore())
        pd_done = es.enter_context(nc.semaphore())
        r_sv = es.enter_context(nc.semaphore())  # src relus on DVE
        r_sa = es.enter_context(nc.semaphore())  # src relus on ACT
        r_sp = es.enter_context(nc.semaphore())  # src relus on Pool
        r_dv = es.enter_context(nc.semaphore())  # dst relus on DVE
        r_da = es.enter_context(nc.semaphore())  # dst relus on ACT
        r_dp = es.enter_context(nc.semaphore())  # dst relus on Pool
        seldone = es.enter_context(nc.semaphore())
        dvedone = es.enter_context(nc.semaphore())
        vchain = es.enter_context(nc.semaphore())
        fin = es.enter_context(nc.semaphore())
        block = es.enter_context(nc.Block())

        emb_bufs = [emb0, emb1]
        pS_bufs = [pS0, pS1]
        pD_bufs = [pD0, pD1]
        ohpS_bufs = [ohpS0, ohpS1]
        ohpD_bufs = [ohpD0, ohpD1]
        mb_bufs = [mb0, mb1]
        qS = [qS0, qS1]
        qD = [qD0, qD1]
        psA = [psA0, psA1]
        psB = [psB0, psB1]
        pload = [ploadA, ploadB]
        mload = [mloadA, mloadB]
        NPRE = 1
        def r_owner(side, c):
            return RELU_OWNER[(side, c)]
        R_SEMS = {(0, "v"): r_sv, (0, "a"): r_sa, (0, "p"): r_sp,
                  (1, "v"): r_dv, (1, "a"): r_da, (1, "p"): r_dp}
        R_PERB = {k: sum(1 for c in range(NCH) if r_owner(k[0], c) == k[1])
                  for k in R_SEMS}
        def r_cum(side, i, c):
            # cumulative count on (side, owner(side, c)) up to and incl (i, c)
            eng = r_owner(side, c)
            n = R_PERB[(side, eng)] * i
            n += sum(1 for cc in range(c + 1) if r_owner(side, cc) == eng)
            return R_SEMS[(side, eng)], n
        def wait_relus_done(eng_obj, side, i):
            for e in ("v", "a", "p"):
                pb = R_PERB[(side, e)]
                if pb:
                    eng_obj.wait_ge(R_SEMS[(side, e)], pb * (i + 1))

        @block.sync
        def _(sync):
            sync.dma_start(out=lhsT8[:], in_=lhsT8_in[:]).then_inc(pre_load, 16)
            # phase-1 shard loads (4 chunks, double buffered)
            for k in range(ECH):
                if k >= 2:
                    sync.wait_ge(ph1_red, k - 1)
                sync.dma_start(
                    out=emb_bufs[k % 2][:],
                    in_=embeds[k * EJ * P:(k + 1) * EJ * P, :].rearrange(
                        "(j p) d -> p j d", p=P
                    ),
                ).then_inc(ph1_load, 16)
            # reload full s from the collective output into S layout
            sync.wait_ge(cc_done, 1)
            sync.dma_start(
                out=S[:].rearrange("p (c u) -> p c u", u=SCOL),
                in_=cc_out[:, :].rearrange("(c p) u -> p c u", p=P),
            ).then_inc(s_loaded, 16)
            # phase-2 streams
            for i in range(nbatch):
                if i >= 2:
                    sync.wait_ge(ps_done, NCH * (i - 1))
                    sync.wait_ge(pd_done, NCH * (i - 1))
                sync.dma_start(out=pS_bufs[i % 2][:], in_=p3_both[i, 0:8]).then_inc(pload[i % 2], 16)
                sync.dma_start(out=pD_bufs[i % 2][:], in_=p3_both[i, 8:16]).then_inc(pload[i % 2], 16)
                if i >= 2:
                    sync.wait_ge(dvedone, i - 1)
                sync.dma_start(out=mb_bufs[i % 2][:], in_=mask_in[i]).then_inc(mload[i % 2], 16)
            sync.wait_ge(fin, 1)
            sync.dma_start(out=y[:], in_=out_sb[:]).then_inc(ydone, 16)

        @block.gpsimd
        def _(gpsimd):
            # stage scaled s_part to DRAM, then AllGather across the 8 cores
            gpsimd.wait_ge(ph1_red, ECH + 1)  # all reduces + scale done
            gpsimd.dma_start(out=cc_in[:, :], in_=s_part[:]).then_inc(cc_staged, 16)
            gpsimd.wait_ge(cc_staged, 16)
            gpsimd.collective_compute(
                "AllGather",
                ALU.bypass,
                replica_groups=[list(range(NCORES))],
                ins=[cc_in[:, :]],
                outs=[cc_out[:, :]],
            ).then_inc(cc_done, 1)

        def relu_vec(eng, i, side, c):
            q = (qS if side == 0 else qD)[(i * NCH + c) % 2]
            ohp = (ohpS_bufs if side == 0 else ohpD_bufs)[i % 2]
            sem, val = r_cum(side, i, c)
            eng.tensor_scalar(
                out=ohp[:, c * CW:(c + 1) * CW],
                in0=q[:],
                scalar1=0.0,
                scalar2=None,
                op0=ALU.max,
            ).then_inc(sem, 1)

        def relu_act(scalar, i, side, c):
            q = (qS if side == 0 else qD)[(i * NCH + c) % 2]
            ohp = (ohpS_bufs if side == 0 else ohpD_bufs)[i % 2]
            sem, val = r_cum(side, i, c)
            scalar.activation(
                out=ohp[:, c * CW:(c + 1) * CW],
                in_=q[:],
                func=ACTF.Relu,
                bias=0.0,
                scale=1.0,
            ).then_inc(sem, 1)

        @block.scalar
        def _(scalar):
            for i in range(nbatch):
                if i >= 2:
                    scalar.wait_ge(seldone, i - 1)  # ohp bufs free
                for side, c in ((1, 0), (1, 1), (1, 2), (1, 3),
                                (0, 0), (0, 1), (0, 2), (0, 3)):
                    if RELU_OWNER[(side, c)] == "a":
                        scalar.wait_ge(
                            (ps_done if side == 0 else pd_done),
                            i * NCH + c + 1,
                        )
                        relu_act(scalar, i, side, c)

        @block.vector
        def _(vector):
            # ---- phase 1: rowsum of this core's shard, scaled ----
            for k in range(ECH):
                vector.wait_ge(ph1_load, 16 * (k + 1))
                vector.tensor_reduce(
                    out=s_part[:, k * EJ:(k + 1) * EJ],
                    in_=emb_bufs[k % 2][:].rearrange("p (j d) -> p j d", d=D),
                    op=ALU.add,
                    axis=mybir.AxisListType.X,
                ).then_inc(ph1_red, 1)
            vector.tensor_scalar(
                out=s_part[:], in0=s_part[:], scalar1=INV_SQ, scalar2=None,
                op0=ALU.mult,
            ).then_inc(ph1_red, 2)
            # ---- build fp16 table from gathered S ----
            vector.wait_ge(s_loaded, 16)
            vector.tensor_scalar(
                out=S_f16[:], in0=S[:], scalar1=1.0, scalar2=None, op0=ALU.mult,
            ).then_inc(tbl_ready, 1)

            # ---- phase 2 ----
            vch = [0]

            def select(i):
                vector.wait_ge(seldone, i + 1)
                vector.wait_ge(mload[i % 2], 16 * (i // 2 + 1))
                if i >= 1:
                    vector.wait_ge(vchain, vch[0])  # dS WAR vs reduce_s(i-1)
                vector.tensor_tensor(
                    out=dS[:], in0=psA[i % 2][:], in1=mb_bufs[i % 2][:, :TPB * CB],
                    op=ALU.mult,
                ).then_inc(vchain, 1)
                vch[0] += 1
                vector.wait_ge(vchain, vch[0])      # dS RAW
                vector.tensor_reduce(
                    out=valS[:, i * TPB:(i + 1) * TPB],
                    in_=dS[:].rearrange("p (t c) -> p t c", c=CB),
                    op=ALU.add,
                    axis=mybir.AxisListType.X,
                ).then_inc(vchain, 1)
                vch[0] += 1
                if i >= 1:
                    vector.wait_ge(dvedone, i)      # dD WAR vs reduce_d(i-1)
                vector.tensor_tensor(
                    out=dD[:], in0=psB[i % 2][:], in1=mb_bufs[i % 2][:, TPB * CB:],
                    op=ALU.mult,
                ).then_inc(vchain, 1)
                vch[0] += 1
                vector.wait_ge(vchain, vch[0])      # dD RAW
                vector.tensor_reduce(
                    out=valD[:, i * TPB:(i + 1) * TPB],
                    in_=dD[:].rearrange("p (t c) -> p t c", c=CB),
                    op=ALU.add,
                    axis=mybir.AxisListType.X,
                ).then_inc(dvedone, 1)

            def relus(i):
                if i >= 2:
                    vector.wait_ge(seldone, i - 1)
                for side in (0, 1):
                    for c in range(NCH):
                        if RELU_OWNER[(side, c)] == "v":
                            vector.wait_ge(
                                (ps_done if side == 0 else pd_done),
                                i * NCH + c + 1,
                            )
                            relu_vec(vector, i, side, c)

            for i in range(nbatch):
                if i >= 1:
                    select(i - 1)
                relus(i)
            select(nbatch - 1)
            vector.wait_ge(vchain, vch[0])
            vector.wait_ge(dvedone, nbatch)
            vector.tensor_tensor(
                out=out_sb[:], in0=valS[:], in1=valD[:], op=ALU.subtract,
            ).then_inc(fin, 1)

        @block.tensor
        def _(tensor):
            tensor.wait_ge(pre_load, 16 * NPRE)
            for i in range(nbatch):
                tensor.wait_ge(pload[i % 2], 32 * (i // 2 + 1))
                for c in range(NCH):
                    q = i * NCH + c
                    if q >= 2:
                        i2, c2 = divmod(q - 2, NCH)
                        sem2, n2 = r_cum(1, i2, c2)
                        tensor.wait_ge(sem2, n2)
                    tensor.matmul(
                        out=qD[q % 2][:],
                        lhsT=lhsT8[:],
                        rhs=pD_bufs[i % 2][:, c * CW:(c + 1) * CW],
                        start=True, stop=True,
                    ).then_inc(pd_done, 1)
                for c in range(NCH):
                    q = i * NCH + c
                    if q >= 2:
                        i2, c2 = divmod(q - 2, NCH)
                        sem2, n2 = r_cum(0, i2, c2)
                        tensor.wait_ge(sem2, n2)  # qS bank free
                    tensor.matmul(
                        out=qS[q % 2][:],
                        lhsT=lhsT8[:],
                        rhs=pS_bufs[i % 2][:, c * CW:(c + 1) * CW],
                        start=True, stop=True,
                    ).then_inc(ps_done, 1)
                if i == 0:
                    tensor.wait_ge(tbl_ready, 1)  # S_f16 ready
                if i >= 2:
                    tensor.wait_ge(dvedone, i - 1)
                for j in range(TPB):
                    if j % CHT == 0:
                        c = j // CHT
                        sem_s, n_s = r_cum(0, i, c)
                        tensor.wait_ge(sem_s, n_s)
                        sem_d, n_d = r_cum(1, i, c)
                        tensor.wait_ge(sem_d, n_d)
                    bs, bd = sched[i * TPB + j]
                    tensor.matmul(
                        out=psA[i % 2][:, j * CB:(j + 1) * CB],
                        lhsT=ohpS_bufs[i % 2][:, j * P:(j + 1) * P],
                        rhs=S_f16[:, bs * CB:(bs + 1) * CB],
                        start=True, stop=True,
                    )
                    mm = tensor.matmul(
                        out=psB[i % 2][:, j * CB:(j + 1) * CB],
                        lhsT=ohpD_bufs[i % 2][:, j * P:(j + 1) * P],
                        rhs=S_f16[:, bd * CB:(bd + 1) * CB],
                        start=True, stop=True,
                    )
                    if j == TPB - 1:
                        mm.then_inc(seldone, 1)

    return nc


def _prep(src_flat, dst_flat):
    E = src_flat.shape[0]
    assert E % NCORES == 0
    Ec = E // NCORES
    NG = NBLK * NBLK

    cores = []
    counts = np.zeros((NCORES, NG), np.int64)
    for i in range(NCORES):
        s = src_flat[i * Ec:(i + 1) * Ec].astype(np.int64)
        d = dst_flat[i * Ec:(i + 1) * Ec].astype(np.int64)
        g = (s >> 12) * NBLK + (d >> 12)
        order = np.argsort(g, kind="stable")
        cores.append((s[order], d[order], g[order], order + i * Ec))
        counts[i] = np.bincount(g, minlength=NG)

    gmax = counts.max(axis=0)
    tiles_per_group = (gmax + P - 1) // P
    n_tiles = int(tiles_per_group.sum())
    n_tiles_p = ((n_tiles + TPB - 1) // TPB) * TPB

    sched = []
    for gi in range(NG):
        sched.extend([(gi // NBLK, gi % NBLK)] * int(tiles_per_group[gi]))
    sched.extend([(0, 0)] * (n_tiles_p - n_tiles))

    slot_base = np.zeros(NG, np.int64)
    np.cumsum(tiles_per_group[:-1] * P, out=slot_base[1:])
    n_slots = n_tiles_p * P
    nbatch = n_tiles_p // TPB

    per_core = []
    for i in range(NCORES):
        s, d, g, orig = cores[i]
        cstart = np.zeros(NG, np.int64)
        np.cumsum(counts[i][:-1], out=cstart[1:])
        within = np.arange(Ec) - cstart[g]
        slot = slot_base[g] + within
        src_s = np.zeros(n_slots, np.int64)
        dst_s = np.zeros(n_slots, np.int64)
        src_s[slot] = s
        dst_s[slot] = d

        def p8(arr):
            pe = (arr & 127).astype(np.float32)
            ae = np.floor(pe / 8.0)
            be = pe - 8.0 * ae
            rows = [ae, ae, np.ones_like(ae), -(ae * ae), -2.0 * (be * be),
                    2.0 * be, -2.0 * (be * be) * 0.0 + np.ones_like(ae),
                    np.ones_like(ae)]
            out = np.stack(rows, axis=0).astype(NPF16)
            return out.reshape(8, nbatch, TPB * P).transpose(1, 0, 2).copy()

        def cmask(arr):
            # [nbatch, P(edge-in-tile), TPB*CB]: one-hot of c_e along CB
            c = ((arr >> 7) & 31).astype(np.int8).reshape(nbatch, TPB, P)
            oh = (c[:, :, :, None] == np.arange(CB, dtype=np.int8)).astype(
                NPBF
            )  # [nbatch, TPB, P(e), CB]
            return np.ascontiguousarray(
                oh.transpose(0, 2, 1, 3).reshape(nbatch, P, TPB * CB)
            )

        per_core.append(
            dict(
                p3_both=np.concatenate([p8(src_s), p8(dst_s)], axis=1),
                mask_both=np.concatenate([cmask(src_s), cmask(dst_s)], axis=2),
                slot=slot,
                orig=orig,
            )
        )
    return per_core, sched, n_tiles_p


def kernel(node_embeds, src_idx, dst_idx):
    node_embeds = np.asarray(node_embeds, dtype=np.float32)
    src_idx = np.asarray(src_idx)
    dst_idx = np.asarray(dst_idx)
    T, E = src_idx.shape
    n_nodes = node_embeds.shape[0]

    src_flat = src_idx.reshape(-1).astype(np.int64)
    dst_flat = dst_idx.reshape(-1).astype(np.int64)
    per_core, sched, n_tiles_p = _prep(src_flat, dst_flat)

    emb_pad = np.zeros((VPAD, D), np.float32)
    emb_pad[:n_nodes] = node_embeds

    iota = np.arange(P, dtype=np.float32)
    a = np.floor(iota / 8.0)
    b = iota - 8.0 * a
    one = np.ones(P, np.float32)
    lhsT8 = np.stack(
        [a, a, -(a * a), one, one, 2.0 * b, -2.0 * (b * b), one]
    ).astype(NPF16)

    nc = _build_nc(n_tiles_p, sched)
    in_maps = []
    for i in range(NCORES):
        pc = per_core[i]
        in_maps.append(
            {
                "embeds": emb_pad[i * NSH:(i + 1) * NSH],
                "p3_both": pc["p3_both"],
                "mask_in": pc["mask_both"],
                "lhsT8": lhsT8,
            }
        )
    res = run_bass_kernel_spmd(nc, in_maps, list(range(NCORES)))

    out_flat = np.zeros(T * E, np.float32)
    for i in range(NCORES):
        pc = per_core[i]
        yv = res.results[i]["y"]
        slot_vals = np.ascontiguousarray(yv.T).reshape(-1)
        out_flat[pc["orig"]] = slot_vals[pc["slot"]]
    return out_flat.reshape(T, E)


# revision 10
# speedup vs baseline: 1.0622x; 1.0622x over previous
"""Trainium2 Bass kernel for nn_DotPred (gnn_message_passing).

score[t, e] = sum_d (x[src] - x[dst]) / sqrt(D)
            = s[src] - s[dst],   s = rowsum(x) / sqrt(D)

Strategy (8 NeuronCores, SPMD):
- Phase 1: rowsum sharded 8 ways (each core reduces 12800 nodes, 6.5MB),
  scaled by 1/sqrt(D), then AllGather (DRAM collective) rebuilds the full
  s table S[128, 800] on every core (node n at partition n & 127,
  column n >> 7).
- Phase 2: per-edge gather of s[src], s[dst] via one-hot matmuls.
  Host pre-sorts each core's edges by (src_block, dst_block) pair
  (block = 4096 nodes = 128 partitions x 32 columns) into 625 groups padded
  to 128-edge tiles (a core-uniform static schedule). Per 128-edge tile:
    PE poly-mm (k=8, bf16):   Q3[p, e] = 1 - (a-a_e)^2 - 2(b-b_e)^2
                              (p = 8a + b; 1 iff p == p_e, else <= 0;
                               all bf16 products are integers <= 256, exact)
    DVE/ACT relu:             OHP[p, e] = relu(Q3) in {0, 1}, bf16
    PE select-mm (fp16):      RT[e, c] = sum_p OHP[p, e] * S_f16[p, c]
    DVE:                      val[e] = sum_c RT[e, c] * mask  (bf16 mask)
  S is selected as fp16 (~2^-11 relative error, far under tolerance).
  Relus are split ACT(5)/Pool(3) per batch; DVE keeps mask-mult+reduce.
- Final: val_src - val_dst on device; host un-permutes.
"""
import math
from contextlib import ExitStack

import numpy as np
import ml_dtypes

import concourse.bass as bass
import concourse.mybir as mybir
from concourse.bass_utils import run_bass_kernel_spmd

P = 128
D = 128
CB = 32             # columns per block
NBLK = 25           # node blocks (4096 nodes each) covering 100096 nodes
N_NODES = 100000
VPAD = 102400       # 8 * 12800 (also 25 * 4096)
NCORES = 8
NSH = VPAD // NCORES    # nodes per core shard (12800)
SCOL = NSH // P         # S columns per core shard (100)
TPB = 16            # tiles per phase-2 batch (one PSUM bank of RT)
CHT = 4             # tiles per poly/relu chunk (one PSUM bank)
NCH = TPB // CHT    # chunks per batch (4)
INV_SQ = 1.0 / math.sqrt(128.0)

F32 = mybir.dt.float32
BF16 = mybir.dt.bfloat16
ALU = mybir.AluOpType
ACTF = mybir.ActivationFunctionType
NPBF = np.dtype(ml_dtypes.bfloat16)
NPF16 = np.dtype(np.float16)

F16 = mybir.dt.float16

# relu chunk ownership: (side, chunk) -> engine 'a' (ACT) / 'p' (Pool) / 'v' (DVE)
RELU_OWNER = {
    (0, 0): "v", (0, 1): "v", (0, 2): "a", (0, 3): "a",
    (1, 0): "a", (1, 1): "a", (1, 2): "a", (1, 3): "a",
}


def _build_nc(n_tiles, sched):
    assert len(sched) == n_tiles and n_tiles % TPB == 0
    nbatch = n_tiles // TPB
    CW = CHT * P         # chunk width in edges (512)
    ECH = 4              # phase-1 chunks
    EJ = SCOL // ECH     # j-columns per phase-1 chunk (25)

    nc = bass.Bass(num_devices=NCORES)
    embeds = nc.declare_dram_parameter("embeds", [NSH, D], F32, isOutput=False)
    p3_both = nc.declare_dram_parameter("p3_both", [nbatch, 16, TPB * P], F16, isOutput=False)
    mask_in = nc.declare_dram_parameter("mask_in", [nbatch, P, 2 * TPB * CB], BF16, isOutput=False)
    lhsT8_in = nc.declare_dram_parameter("lhsT8", [8, P], F16, isOutput=False)
    y = nc.declare_dram_parameter("y", [P, n_tiles], F32, isOutput=True)

    cc_in = nc.dram_tensor("cc_in", [P, SCOL], F32)
    cc_out = nc.dram_tensor("cc_out", [NCORES * P, SCOL], F32)

    es = ExitStack()
    with es:
        emb0 = es.enter_context(nc.sbuf_tensor([P, EJ * D], F32))
        emb1 = es.enter_context(nc.sbuf_tensor([P, EJ * D], F32))
        s_part = es.enter_context(nc.sbuf_tensor([P, SCOL], F32))
        S = es.enter_context(nc.sbuf_tensor([P, NBLK * CB], F32))
        S_f16 = es.enter_context(nc.sbuf_tensor([P, NBLK * CB], F16))
        lhsT8 = es.enter_context(nc.sbuf_tensor([8, P], F16))
        pS0 = es.enter_context(nc.sbuf_tensor([8, TPB * P], F16))
        pS1 = es.enter_context(nc.sbuf_tensor([8, TPB * P], F16))
        pD0 = es.enter_context(nc.sbuf_tensor([8, TPB * P], F16))
        pD1 = es.enter_context(nc.sbuf_tensor([8, TPB * P], F16))
        ohpS0 = es.enter_context(nc.sbuf_tensor([P, TPB * P], F16))
        ohpS1 = es.enter_context(nc.sbuf_tensor([P, TPB * P], F16))
        ohpD0 = es.enter_context(nc.sbuf_tensor([P, TPB * P], F16))
        ohpD1 = es.enter_context(nc.sbuf_tensor([P, TPB * P], F16))
        mb0 = es.enter_context(nc.sbuf_tensor([P, 2 * TPB * CB], BF16))
        mb1 = es.enter_context(nc.sbuf_tensor([P, 2 * TPB * CB], BF16))
        dS = es.enter_context(nc.sbuf_tensor([P, TPB * CB], F32))
        dD = es.enter_context(nc.sbuf_tensor([P, TPB * CB], F32))
        valS = es.enter_context(nc.sbuf_tensor([P, n_tiles], F32))
        valD = es.enter_context(nc.sbuf_tensor([P, n_tiles], F32))
        out_sb = es.enter_context(nc.sbuf_tensor([P, n_tiles], F32))
        qS0 = es.enter_context(nc.psum_tensor([P, CW], F32))
        qS1 = es.enter_context(nc.psum_tensor([P, CW], F32))
        qD0 = es.enter_context(nc.psum_tensor([P, CW], F32))
        qD1 = es.enter_context(nc.psum_tensor([P, CW], F32))
        psA0 = es.enter_context(nc.psum_tensor([P, TPB * CB], F32))
        psA1 = es.enter_context(nc.psum_tensor([P, TPB * CB], F32))
        psB0 = es.enter_context(nc.psum_tensor([P, TPB * CB], F32))
        psB1 = es.enter_context(nc.psum_tensor([P, TPB * CB], F32))
        ph1_load = es.enter_context(nc.semaphore())
        ph1_red = es.enter_context(nc.semaphore())
        cc_staged = es.enter_context(nc.semaphore())
        cc_done = es.enter_context(nc.semaphore())
        s_loaded = es.enter_context(nc.semaphore())
        tbl_ready = es.enter_context(nc.semaphore())
        pre_load = es.enter_context(nc.semaphore())
        ploadA = es.enter_context(nc.semaphore())
        ploadB = es.enter_context(nc.semaphore())
        mloadA = es.enter_context(nc.semaphore())
        mloadB = es.enter_context(nc.semaphore())
        ydone = es.enter_context(nc.semaphore())
        ps_done = es.enter_context(nc.semaphore())
        pd_done = es.enter_context(nc.semaphore())
        r_sv = es.enter_context(nc.semaphore())  # src relus on DVE
        r_sa = es.enter_context(nc.semaphore())  # src relus on ACT
        r_sp = es.enter_context(nc.semaphore())  # src relus on Pool
        r_dv = es.enter_context(nc.semaphore())  # dst relus on DVE
        r_da = es.enter_context(nc.semaphore())  # dst relus on ACT
        r_dp = es.enter_context(nc.semaphore())  # dst relus on Pool
        seldone = es.enter_context(nc.semaphore())
        dvedone = es.enter_context(nc.semaphore())
        vchain = es.enter_context(nc.semaphore())
        fin = es.enter_context(nc.semaphore())
        block = es.enter_context(nc.Block())

        emb_bufs = [emb0, emb1]
        pS_bufs = [pS0, pS1]
        pD_bufs = [pD0, pD1]
        ohpS_bufs = [ohpS0, ohpS1]
        ohpD_bufs = [ohpD0, ohpD1]
        mb_bufs = [mb0, mb1]
        qS = [qS0, qS1]
        qD = [qD0, qD1]
        psA = [psA0, psA1]
        psB = [psB0, psB1]
        pload = [ploadA, ploadB]
        mload = [mloadA, mloadB]
        NPRE = 1
        def r_owner(side, c):
            return RELU_OWNER[(side, c)]
        R_SEMS = {(0, "v"): r_sv, (0, "a"): r_sa, (0, "p"): r_sp,
                  (1, "v"): r_dv, (1, "a"): r_da, (1, "p"): r_dp}
        R_PERB = {k: sum(1 for c in range(NCH) if r_owner(k[0], c) == k[1])
                  for k in R_SEMS}
        def r_cum(side, i, c):
            # cumulative count on (side, owner(side, c)) up to and incl (i, c)
            eng = r_owner(side, c)
            n = R_PERB[(side, eng)] * i
            n += sum(1 for cc in range(c + 1) if r_owner(side, cc) == eng)
            return R_SEMS[(side, eng)], n
        def wait_relus_done(eng_obj, side, i):
            for e in ("v", "a", "p"):
                pb = R_PERB[(side, e)]
                if pb:
                    eng_obj.wait_ge(R_SEMS[(side, e)], pb * (i + 1))

        @block.sync
        def _(sync):
            sync.dma_start(out=lhsT8[:], in_=lhsT8_in[:]).then_inc(pre_load, 16)
            # phase-1 shard loads (4 chunks, double buffered)
            for k in range(ECH):
                if k >= 2:
                    sync.wait_ge(ph1_red, k - 1)
                sync.dma_start(
                    out=emb_bufs[k % 2][:],
                    in_=embeds[k * EJ * P:(k + 1) * EJ * P, :].rearrange(
                        "(j p) d -> p j d", p=P
                    ),
                ).then_inc(ph1_load, 16)
            # reload full s from the collective output into S layout
            sync.wait_ge(cc_done, 1)
            sync.dma_start(
                out=S[:].rearrange("p (c u) -> p c u", u=SCOL),
                in_=cc_out[:, :].rearrange("(c p) u -> p c u", p=P),
            ).then_inc(s_loaded, 16)
            # phase-2 streams
            for i in range(nbatch):
                if i >= 2:
                    sync.wait_ge(ps_done, NCH * (i - 1))
                    sync.wait_ge(pd_done, NCH * (i - 1))
                sync.dma_start(out=pS_bufs[i % 2][:], in_=p3_both[i, 0:8]).then_inc(pload[i % 2], 16)
                sync.dma_start(out=pD_bufs[i % 2][:], in_=p3_both[i, 8:16]).then_inc(pload[i % 2], 16)
                if i >= 2:
                    sync.wait_ge(dvedone, i - 1)
                sync.dma_start(out=mb_bufs[i % 2][:], in_=mask_in[i]).then_inc(mload[i % 2], 16)
            sync.wait_ge(fin, 1)
            sync.dma_start(out=y[:], in_=out_sb[:]).then_inc(ydone, 16)

        @block.gpsimd
        def _(gpsimd):
            # stage scaled s_part to DRAM, then AllGather across the 8 cores
            gpsimd.wait_ge(ph1_red, ECH + 1)  # all reduces + scale done
            gpsimd.dma_start(out=cc_in[:, :], in_=s_part[:]).then_inc(cc_staged, 16)
            gpsimd.wait_ge(cc_staged, 16)
            gpsimd.collective_compute(
                "AllGather",
                ALU.bypass,
                replica_groups=[list(range(NCORES))],
                ins=[cc_in[:, :]],
                outs=[cc_out[:, :]],
            ).then_inc(cc_done, 1)

        def relu_vec(eng, i, side, c):
            q = (qS if side == 0 else qD)[(i * NCH + c) % 2]
            ohp = (ohpS_bufs if side == 0 else ohpD_bufs)[i % 2]
            sem, val = r_cum(side, i, c)
            eng.tensor_scalar(
                out=ohp[:, c * CW:(c + 1) * CW],
                in0=q[:],
                scalar1=0.0,
                scalar2=None,
                op0=ALU.max,
            ).then_inc(sem, 1)

        def relu_act(scalar, i, side, c):
            q = (qS if side == 0 else qD)[(i * NCH + c) % 2]
            ohp = (ohpS_bufs if side == 0 else ohpD_bufs)[i % 2]
            sem, val = r_cum(side, i, c)
            scalar.activation(
                out=ohp[:, c * CW:(c + 1) * CW],
                in_=q[:],
                func=ACTF.Relu,
                bias=0.0,
                scale=1.0,
            ).then_inc(sem, 1)

        @block.scalar
        def _(scalar):
            for i in range(nbatch):
                if i >= 2:
                    scalar.wait_ge(seldone, i - 1)  # ohp bufs free
                for side, c in ((1, 0), (1, 1), (1, 2), (0, 2),
                                (1, 3), (0, 3), (0, 0), (0, 1)):
                    if RELU_OWNER[(side, c)] == "a":
                        scalar.wait_ge(
                            (ps_done if side == 0 else pd_done),
                            i * NCH + c + 1,
                        )
                        relu_act(scalar, i, side, c)

        @block.vector
        def _(vector):
            # ---- phase 1: rowsum of this core's shard, scaled ----
            for k in range(ECH):
                vector.wait_ge(ph1_load, 16 * (k + 1))
                vector.tensor_reduce(
                    out=s_part[:, k * EJ:(k + 1) * EJ],
                    in_=emb_bufs[k % 2][:].rearrange("p (j d) -> p j d", d=D),
                    op=ALU.add,
                    axis=mybir.AxisListType.X,
                ).then_inc(ph1_red, 1)
            vector.tensor_scalar(
                out=s_part[:], in0=s_part[:], scalar1=INV_SQ, scalar2=None,
                op0=ALU.mult,
            ).then_inc(ph1_red, 2)
            # ---- build fp16 table from gathered S ----
            vector.wait_ge(s_loaded, 16)
            vector.tensor_scalar(
                out=S_f16[:], in0=S[:], scalar1=1.0, scalar2=None, op0=ALU.mult,
            ).then_inc(tbl_ready, 1)

            # ---- phase 2 ----
            vch = [0]

            def select(i):
                vector.wait_ge(seldone, i + 1)
                vector.wait_ge(mload[i % 2], 16 * (i // 2 + 1))
                if i >= 1:
                    vector.wait_ge(vchain, vch[0])  # dS WAR vs reduce_s(i-1)
                vector.tensor_tensor(
                    out=dS[:], in0=psA[i % 2][:], in1=mb_bufs[i % 2][:, :TPB * CB],
                    op=ALU.mult,
                ).then_inc(vchain, 1)
                vch[0] += 1
                vector.wait_ge(vchain, vch[0])      # dS RAW
                vector.tensor_reduce(
                    out=valS[:, i * TPB:(i + 1) * TPB],
                    in_=dS[:].rearrange("p (t c) -> p t c", c=CB),
                    op=ALU.add,
                    axis=mybir.AxisListType.X,
                ).then_inc(vchain, 1)
                vch[0] += 1
                if i >= 1:
                    vector.wait_ge(dvedone, i)      # dD WAR vs reduce_d(i-1)
                vector.tensor_tensor(
                    out=dD[:], in0=psB[i % 2][:], in1=mb_bufs[i % 2][:, TPB * CB:],
                    op=ALU.mult,
                ).then_inc(vchain, 1)
                vch[0] += 1
                vector.wait_ge(vchain, vch[0])      # dD RAW
                vector.tensor_reduce(
                    out=valD[:, i * TPB:(i + 1) * TPB],
                    in_=dD[:].rearrange("p (t c) -> p t c", c=CB),
                    op=ALU.add,
                    axis=mybir.AxisListType.X,
                ).then_inc(dvedone, 1)

            def relus(i):
                if i >= 2:
                    vector.wait_ge(seldone, i - 1)
                for side in (0, 1):
                    for c in range(NCH):
                        if RELU_OWNER[(side, c)] == "v":
                            vector.wait_ge(
                                (ps_done if side == 0 else pd_done),
                                i * NCH + c + 1,
                            )
                            relu_vec(vector, i, side, c)

            for i in range(nbatch):
                relus(i)
                if i >= 1:
                    select(i - 1)
            select(nbatch - 1)
            vector.wait_ge(vchain, vch[0])
            vector.wait_ge(dvedone, nbatch)
            vector.tensor_tensor(
                out=out_sb[:], in0=valS[:], in1=valD[:], op=ALU.subtract,
            ).then_inc(fin, 1)

        @block.tensor
        def _(tensor):
            tensor.wait_ge(pre_load, 16 * NPRE)
            for i in range(nbatch):
                tensor.wait_ge(pload[i % 2], 32 * (i // 2 + 1))
                for c in range(NCH):
                    q = i * NCH + c
                    if q >= 2:
                        i2, c2 = divmod(q - 2, NCH)
                        sem2, n2 = r_cum(1, i2, c2)
                        tensor.wait_ge(sem2, n2)
                    tensor.matmul(
                        out=qD[q % 2][:],
                        lhsT=lhsT8[:],
                        rhs=pD_bufs[i % 2][:, c * CW:(c + 1) * CW],
                        start=True, stop=True,
                    ).then_inc(pd_done, 1)
                for c in range(NCH):
                    q = i * NCH + c
                    if q >= 2:
                        i2, c2 = divmod(q - 2, NCH)
                        sem2, n2 = r_cum(0, i2, c2)
                        tensor.wait_ge(sem2, n2)  # qS bank free
                    tensor.matmul(
                        out=qS[q % 2][:],
                        lhsT=lhsT8[:],
                        rhs=pS_bufs[i % 2][:, c * CW:(c + 1) * CW],
                        start=True, stop=True,
                    ).then_inc(ps_done, 1)
                if i == 0:
                    tensor.wait_ge(tbl_ready, 1)  # S_f16 ready
                if i >= 2:
                    tensor.wait_ge(dvedone, i - 1)
                for j in range(TPB):
                    if j % CHT == 0:
                        c = j // CHT
                        sem_s, n_s = r_cum(0, i, c)
                        tensor.wait_ge(sem_s, n_s)
                        sem_d, n_d = r_cum(1, i, c)
                        tensor.wait_ge(sem_d, n_d)
                    bs, bd = sched[i * TPB + j]
                    tensor.matmul(
                        out=psA[i % 2][:, j * CB:(j + 1) * CB],
                        lhsT=ohpS_bufs[i % 2][:, j * P:(j + 1) * P],
                        rhs=S_f16[:, bs * CB:(bs + 1) * CB],
                        start=True, stop=True,
                    )
                    mm = tensor.matmul(
                        out=psB[i % 2][:, j * CB:(j + 1) * CB],
                        lhsT=ohpD_bufs[i % 2][:, j * P:(j + 1) * P],
                        rhs=S_f16[:, bd * CB:(bd + 1) * CB],
                        start=True, stop=True,
                    )
                    if j == TPB - 1:
                        mm.then_inc(seldone, 1)

    return nc


def _prep(src_flat, dst_flat):
    E = src_flat.shape[0]
    assert E % NCORES == 0
    Ec = E // NCORES
    NG = NBLK * NBLK

    cores = []
    counts = np.zeros((NCORES, NG), np.int64)
    for i in range(NCORES):
        s = src_flat[i * Ec:(i + 1) * Ec].astype(np.int64)
        d = dst_flat[i * Ec:(i + 1) * Ec].astype(np.int64)
        g = (s >> 12) * NBLK + (d >> 12)
        order = np.argsort(g, kind="stable")
        cores.append((s[order], d[order], g[order], order + i * Ec))
        counts[i] = np.bincount(g, minlength=NG)

    gmax = counts.max(axis=0)
    tiles_per_group = (gmax + P - 1) // P
    n_tiles = int(tiles_per_group.sum())
    n_tiles_p = ((n_tiles + TPB - 1) // TPB) * TPB

    sched = []
    for gi in range(NG):
        sched.extend([(gi // NBLK, gi % NBLK)] * int(tiles_per_group[gi]))
    sched.extend([(0, 0)] * (n_tiles_p - n_tiles))

    slot_base = np.zeros(NG, np.int64)
    np.cumsum(tiles_per_group[:-1] * P, out=slot_base[1:])
    n_slots = n_tiles_p * P
    nbatch = n_tiles_p // TPB

    per_core = []
    for i in range(NCORES):
        s, d, g, orig = cores[i]
        cstart = np.zeros(NG, np.int64)
        np.cumsum(counts[i][:-1], out=cstart[1:])
        within = np.arange(Ec) - cstart[g]
        slot = slot_base[g] + within
        src_s = np.zeros(n_slots, np.int64)
        dst_s = np.zeros(n_slots, np.int64)
        src_s[slot] = s
        dst_s[slot] = d

        def p8(arr):
            pe = (arr & 127).astype(np.float32)
            ae = np.floor(pe / 8.0)
            be = pe - 8.0 * ae
            rows = [ae, ae, np.ones_like(ae), -(ae * ae), -2.0 * (be * be),
                    2.0 * be, -2.0 * (be * be) * 0.0 + np.ones_like(ae),
                    np.ones_like(ae)]
            out = np.stack(rows, axis=0).astype(NPF16)
            return out.reshape(8, nbatch, TPB * P).transpose(1, 0, 2).copy()

        def cmask(arr):
            # [nbatch, P(edge-in-tile), TPB*CB]: one-hot of c_e along CB
            c = ((arr >> 7) & 31).astype(np.int8).reshape(nbatch, TPB, P)
            oh = (c[:, :, :, None] == np.arange(CB, dtype=np.int8)).astype(
                NPBF
            )  # [nbatch, TPB, P(e), CB]
            return np.ascontiguousarray(
                oh.transpose(0, 2, 1, 3).reshape(nbatch, P, TPB * CB)
            )

        per_core.append(
            dict(
                p3_both=np.concatenate([p8(src_s), p8(dst_s)], axis=1),
                mask_both=np.concatenate([cmask(src_s), cmask(dst_s)], axis=2),
                slot=slot,
                orig=orig,
            )
        )
    return per_core, sched, n_tiles_p


def kernel(node_embeds, src_idx, dst_idx):
    node_embeds = np.asarray(node_embeds, dtype=np.float32)
    src_idx = np.asarray(src_idx)
    dst_idx = np.asarray(dst_idx)
    T, E = src_idx.shape
    n_nodes = node_embeds.shape[0]

    src_flat = src_idx.reshape(-1).astype(np.int64)
    dst_flat = dst_idx.reshape(-1).astype(np.int64)
    per_core, sched, n_tiles_p = _prep(src_flat, dst_flat)

    emb_pad = np.zeros((VPAD, D), np.float32)
    emb_pad[:n_nodes] = node_embeds

    iota = np.arange(P, dtype=np.float32)
    a = np.floor(iota / 8.0)
    b = iota - 8.0 * a
    one = np.ones(P, np.float32)
    lhsT8 = np.stack(
        [a, a, -(a * a), one, one, 2.0 * b, -2.0 * (b * b), one]
    ).astype(NPF16)

    nc = _build_nc(n_tiles_p, sched)
    in_maps = []
    for i in range(NCORES):
        pc = per_core[i]
        in_maps.append(
            {
                "embeds": emb_pad[i * NSH:(i + 1) * NSH],
                "p3_both": pc["p3_both"],
                "mask_in": pc["mask_both"],
                "lhsT8": lhsT8,
            }
        )
    res = run_bass_kernel_spmd(nc, in_maps, list(range(NCORES)))

    out_flat = np.zeros(T * E, np.float32)
    for i in range(NCORES):
        pc = per_core[i]
        yv = res.results[i]["y"]
        slot_vals = np.ascontiguousarray(yv.T).reshape(-1)
        out_flat[pc["orig"]] = slot_vals[pc["slot"]]
    return out_flat.reshape(T, E)


# revision 11
# speedup vs baseline: 1.1614x; 1.0933x over previous
"""Trainium2 Bass kernel for nn_DotPred (gnn_message_passing).

score[t, e] = sum_d (x[src] - x[dst]) / sqrt(D)
            = s[src] - s[dst],   s = rowsum(x) / sqrt(D)

Strategy (8 NeuronCores, SPMD):
- Phase 1: rowsum sharded 8 ways (each core reduces 12800 nodes, 6.5MB),
  scaled by 1/sqrt(D), then AllGather (DRAM collective) rebuilds the full
  s table S[128, 800] on every core (node n at partition n & 127,
  column n >> 7).
- Phase 2: per-edge gather of s[src], s[dst] via one-hot matmuls.
  Host pre-sorts each core's edges by (src_block, dst_block) pair
  (block = 4096 nodes = 128 partitions x 32 columns) into 625 groups padded
  to 128-edge tiles (a core-uniform static schedule). Per 128-edge tile:
    PE poly-mm (k=8, bf16):   Q3[p, e] = 1 - (a-a_e)^2 - 2(b-b_e)^2
                              (p = 8a + b; 1 iff p == p_e, else <= 0;
                               all bf16 products are integers <= 256, exact)
    DVE/ACT relu:             OHP[p, e] = relu(Q3) in {0, 1}, bf16
    PE select-mm (fp16):      RT[e, c] = sum_p OHP[p, e] * S_f16[p, c]
    DVE:                      val[e] = sum_c RT[e, c] * mask  (bf16 mask)
  S is selected as fp16 (~2^-11 relative error, far under tolerance).
  Relus are split ACT(5)/Pool(3) per batch; DVE keeps mask-mult+reduce.
- Final: val_src - val_dst on device; host un-permutes.
"""
import math
from contextlib import ExitStack

import numpy as np
import ml_dtypes

import concourse.bass as bass
import concourse.mybir as mybir
from concourse.bass_utils import run_bass_kernel_spmd

P = 128
D = 128
CB = 32             # columns per block
NBLK = 25           # node blocks (4096 nodes each) covering 100096 nodes
N_NODES = 100000
VPAD = 102400       # 8 * 12800 (also 25 * 4096)
NCORES = 8
NSH = VPAD // NCORES    # nodes per core shard (12800)
SCOL = NSH // P         # S columns per core shard (100)
TPB = 16            # tiles per phase-2 batch (one PSUM bank of RT)
CHT = 4             # tiles per poly/relu chunk (one PSUM bank)
NCH = TPB // CHT    # chunks per batch (4)
INV_SQ = 1.0 / math.sqrt(128.0)

F32 = mybir.dt.float32
BF16 = mybir.dt.bfloat16
ALU = mybir.AluOpType
ACTF = mybir.ActivationFunctionType
NPBF = np.dtype(ml_dtypes.bfloat16)
NPF16 = np.dtype(np.float16)

F16 = mybir.dt.float16

# relu chunk ownership: (side, chunk) -> engine 'a' (ACT) / 'p' (Pool) / 'v' (DVE)
RELU_OWNER = {
    (0, 0): "v", (0, 1): "v", (0, 2): "a", (0, 3): "a",
    (1, 0): "a", (1, 1): "a", (1, 2): "a", (1, 3): "a",
}


def _build_nc(n_tiles, sched):
    assert len(sched) == n_tiles and n_tiles % TPB == 0
    nbatch = n_tiles // TPB
    CW = CHT * P         # chunk width in edges (512)
    ECH = 4              # phase-1 chunks
    EJ = SCOL // ECH     # j-columns per phase-1 chunk (25)

    nc = bass.Bass(num_devices=NCORES)
    embeds = nc.declare_dram_parameter("embeds", [NSH, D], F32, isOutput=False)
    p3_both = nc.declare_dram_parameter("p3_both", [nbatch, 16, TPB * P], F16, isOutput=False)
    mask_in = nc.declare_dram_parameter("mask_in", [nbatch, P, 2 * TPB * CB], BF16, isOutput=False)
    lhsT8_in = nc.declare_dram_parameter("lhsT8", [8, P], F16, isOutput=False)
    y = nc.declare_dram_parameter("y", [P, n_tiles], F32, isOutput=True)

    cc_in = nc.dram_tensor("cc_in", [P, SCOL], F32)
    cc_out = nc.dram_tensor("cc_out", [NCORES * P, SCOL], F32)

    es = ExitStack()
    with es:
        emb0 = es.enter_context(nc.sbuf_tensor([P, EJ * D], F32))
        emb1 = es.enter_context(nc.sbuf_tensor([P, EJ * D], F32))
        s_part = es.enter_context(nc.sbuf_tensor([P, SCOL], F32))
        S = es.enter_context(nc.sbuf_tensor([P, NBLK * CB], F32))
        S_f16 = es.enter_context(nc.sbuf_tensor([P, NBLK * CB], F16))
        lhsT8 = es.enter_context(nc.sbuf_tensor([8, P], F16))
        pS0 = es.enter_context(nc.sbuf_tensor([8, TPB * P], F16))
        pS1 = es.enter_context(nc.sbuf_tensor([8, TPB * P], F16))
        pD0 = es.enter_context(nc.sbuf_tensor([8, TPB * P], F16))
        pD1 = es.enter_context(nc.sbuf_tensor([8, TPB * P], F16))
        ohpS0 = es.enter_context(nc.sbuf_tensor([P, TPB * P], F16))
        ohpS1 = es.enter_context(nc.sbuf_tensor([P, TPB * P], F16))
        ohpD0 = es.enter_context(nc.sbuf_tensor([P, TPB * P], F16))
        ohpD1 = es.enter_context(nc.sbuf_tensor([P, TPB * P], F16))
        mb0 = es.enter_context(nc.sbuf_tensor([P, 2 * TPB * CB], BF16))
        mb1 = es.enter_context(nc.sbuf_tensor([P, 2 * TPB * CB], BF16))
        dS = es.enter_context(nc.sbuf_tensor([P, TPB * CB], F32))
        dD = es.enter_context(nc.sbuf_tensor([P, TPB * CB], F32))
        valS = es.enter_context(nc.sbuf_tensor([P, n_tiles], F32))
        valD = es.enter_context(nc.sbuf_tensor([P, n_tiles], F32))
        out_sb = es.enter_context(nc.sbuf_tensor([P, n_tiles], F32))
        qS0 = es.enter_context(nc.psum_tensor([P, CW], F32))
        qS1 = es.enter_context(nc.psum_tensor([P, CW], F32))
        qD0 = es.enter_context(nc.psum_tensor([P, CW], F32))
        qD1 = es.enter_context(nc.psum_tensor([P, CW], F32))
        psA0 = es.enter_context(nc.psum_tensor([P, TPB * CB], F32))
        psA1 = es.enter_context(nc.psum_tensor([P, TPB * CB], F32))
        psB0 = es.enter_context(nc.psum_tensor([P, TPB * CB], F32))
        psB1 = es.enter_context(nc.psum_tensor([P, TPB * CB], F32))
        ph1_load = es.enter_context(nc.semaphore())
        ph1_red = es.enter_context(nc.semaphore())
        cc_staged = es.enter_context(nc.semaphore())
        cc_done = es.enter_context(nc.semaphore())
        s_loaded = es.enter_context(nc.semaphore())
        tbl_ready = es.enter_context(nc.semaphore())
        pre_load = es.enter_context(nc.semaphore())
        ploadA = es.enter_context(nc.semaphore())
        ploadB = es.enter_context(nc.semaphore())
        mloadA = es.enter_context(nc.semaphore())
        mloadB = es.enter_context(nc.semaphore())
        ydone = es.enter_context(nc.semaphore())
        ps_done = es.enter_context(nc.semaphore())
        pd_done = es.enter_context(nc.semaphore())
        r_sv = es.enter_context(nc.semaphore())  # src relus on DVE
        r_sa = es.enter_context(nc.semaphore())  # src relus on ACT
        r_sp = es.enter_context(nc.semaphore())  # src relus on Pool
        r_dv = es.enter_context(nc.semaphore())  # dst relus on DVE
        r_da = es.enter_context(nc.semaphore())  # dst relus on ACT
        r_dp = es.enter_context(nc.semaphore())  # dst relus on Pool
        seldone = es.enter_context(nc.semaphore())
        dvedone = es.enter_context(nc.semaphore())
        vchain = es.enter_context(nc.semaphore())
        fin = es.enter_context(nc.semaphore())
        block = es.enter_context(nc.Block())

        emb_bufs = [emb0, emb1]
        pS_bufs = [pS0, pS1]
        pD_bufs = [pD0, pD1]
        ohpS_bufs = [ohpS0, ohpS1]
        ohpD_bufs = [ohpD0, ohpD1]
        mb_bufs = [mb0, mb1]
        qS = [qS0, qS1]
        qD = [qD0, qD1]
        psA = [psA0, psA1]
        psB = [psB0, psB1]
        pload = [ploadA, ploadB]
        mload = [mloadA, mloadB]
        NPRE = 1
        def r_owner(side, c):
            return RELU_OWNER[(side, c)]
        R_SEMS = {(0, "v"): r_sv, (0, "a"): r_sa, (0, "p"): r_sp,
                  (1, "v"): r_dv, (1, "a"): r_da, (1, "p"): r_dp}
        R_PERB = {k: sum(1 for c in range(NCH) if r_owner(k[0], c) == k[1])
                  for k in R_SEMS}
        def r_cum(side, i, c):
            # cumulative count on (side, owner(side, c)) up to and incl (i, c)
            eng = r_owner(side, c)
            n = R_PERB[(side, eng)] * i
            n += sum(1 for cc in range(c + 1) if r_owner(side, cc) == eng)
            return R_SEMS[(side, eng)], n
        def wait_relus_done(eng_obj, side, i):
            for e in ("v", "a", "p"):
                pb = R_PERB[(side, e)]
                if pb:
                    eng_obj.wait_ge(R_SEMS[(side, e)], pb * (i + 1))

        @block.sync
        def _(sync):
            sync.dma_start(out=lhsT8[:], in_=lhsT8_in[:]).then_inc(pre_load, 16)
            # phase-1 shard loads (4 chunks, double buffered)
            for k in range(ECH):
                if k >= 2:
                    sync.wait_ge(ph1_red, k - 1)
                sync.dma_start(
                    out=emb_bufs[k % 2][:],
                    in_=embeds[k * EJ * P:(k + 1) * EJ * P, :].rearrange(
                        "(j p) d -> p j d", p=P
                    ),
                ).then_inc(ph1_load, 16)
            # reload full s from the collective output into S layout
            sync.wait_ge(cc_done, 1)
            sync.dma_start(
                out=S[:].rearrange("p (c u) -> p c u", u=SCOL),
                in_=cc_out[:, :].rearrange("(c p) u -> p c u", p=P),
            ).then_inc(s_loaded, 16)
            # phase-2 streams
            for i in range(nbatch):
                if i >= 2:
                    sync.wait_ge(ps_done, NCH * (i - 1))
                    sync.wait_ge(pd_done, NCH * (i - 1))
                sync.dma_start(out=pS_bufs[i % 2][:], in_=p3_both[i, 0:8]).then_inc(pload[i % 2], 16)
                sync.dma_start(out=pD_bufs[i % 2][:], in_=p3_both[i, 8:16]).then_inc(pload[i % 2], 16)
                if i >= 2:
                    sync.wait_ge(dvedone, i - 1)
                sync.dma_start(out=mb_bufs[i % 2][:], in_=mask_in[i]).then_inc(mload[i % 2], 16)
            sync.wait_ge(fin, 1)
            sync.dma_start(out=y[:], in_=out_sb[:]).then_inc(ydone, 16)

        @block.gpsimd
        def _(gpsimd):
            # stage scaled s_part to DRAM, then AllGather across the 8 cores
            gpsimd.wait_ge(ph1_red, ECH + 1)  # all reduces + scale done
            gpsimd.dma_start(out=cc_in[:, :], in_=s_part[:]).then_inc(cc_staged, 16)
            gpsimd.wait_ge(cc_staged, 16)
            gpsimd.collective_compute(
                "AllGather",
                ALU.bypass,
                replica_groups=[list(range(NCORES))],
                ins=[cc_in[:, :]],
                outs=[cc_out[:, :]],
            ).then_inc(cc_done, 1)

        def relu_vec(eng, i, side, c):
            q = (qS if side == 0 else qD)[(i * NCH + c) % 2]
            ohp = (ohpS_bufs if side == 0 else ohpD_bufs)[i % 2]
            sem, val = r_cum(side, i, c)
            eng.tensor_scalar(
                out=ohp[:, c * CW:(c + 1) * CW],
                in0=q[:],
                scalar1=0.0,
                scalar2=None,
                op0=ALU.max,
            ).then_inc(sem, 1)

        def relu_act(scalar, i, side, c):
            q = (qS if side == 0 else qD)[(i * NCH + c) % 2]
            ohp = (ohpS_bufs if side == 0 else ohpD_bufs)[i % 2]
            sem, val = r_cum(side, i, c)
            scalar.activation(
                out=ohp[:, c * CW:(c + 1) * CW],
                in_=q[:],
                func=ACTF.Relu,
                bias=0.0,
                scale=1.0,
            ).then_inc(sem, 1)

        @block.scalar
        def _(scalar):
            for i in range(nbatch):
                if i >= 2:
                    scalar.wait_ge(seldone, i - 1)  # ohp bufs free
                for side, c in ((1, 0), (1, 1), (1, 2), (1, 3),
                                (0, 0), (0, 1), (0, 2), (0, 3)):
                    if RELU_OWNER[(side, c)] == "a":
                        scalar.wait_ge(
                            (ps_done if side == 0 else pd_done),
                            i * NCH + c + 1,
                        )
                        relu_act(scalar, i, side, c)

        @block.vector
        def _(vector):
            # ---- phase 1: rowsum of this core's shard, scaled ----
            for k in range(ECH):
                vector.wait_ge(ph1_load, 16 * (k + 1))
                vector.tensor_reduce(
                    out=s_part[:, k * EJ:(k + 1) * EJ],
                    in_=emb_bufs[k % 2][:].rearrange("p (j d) -> p j d", d=D),
                    op=ALU.add,
                    axis=mybir.AxisListType.X,
                ).then_inc(ph1_red, 1)
            vector.tensor_scalar(
                out=s_part[:], in0=s_part[:], scalar1=INV_SQ, scalar2=None,
                op0=ALU.mult,
            ).then_inc(ph1_red, 2)
            # ---- build fp16 table from gathered S ----
            vector.wait_ge(s_loaded, 16)
            vector.tensor_scalar(
                out=S_f16[:], in0=S[:], scalar1=1.0, scalar2=None, op0=ALU.mult,
            ).then_inc(tbl_ready, 1)

            # ---- phase 2 ----
            vch = [0]

            def select(i):
                vector.wait_ge(seldone, i + 1)
                vector.wait_ge(mload[i % 2], 16 * (i // 2 + 1))
                if i >= 1:
                    vector.wait_ge(vchain, vch[0])  # dS WAR vs reduce_s(i-1)
                vector.tensor_tensor(
                    out=dS[:], in0=psA[i % 2][:], in1=mb_bufs[i % 2][:, :TPB * CB],
                    op=ALU.mult,
                ).then_inc(vchain, 1)
                vch[0] += 1
                vector.wait_ge(vchain, vch[0])      # dS RAW
                vector.tensor_reduce(
                    out=valS[:, i * TPB:(i + 1) * TPB],
                    in_=dS[:].rearrange("p (t c) -> p t c", c=CB),
                    op=ALU.add,
                    axis=mybir.AxisListType.X,
                ).then_inc(vchain, 1)
                vch[0] += 1
                if i >= 1:
                    vector.wait_ge(dvedone, i)      # dD WAR vs reduce_d(i-1)
                vector.tensor_tensor(
                    out=dD[:], in0=psB[i % 2][:], in1=mb_bufs[i % 2][:, TPB * CB:],
                    op=ALU.mult,
                ).then_inc(vchain, 1)
                vch[0] += 1
                vector.wait_ge(vchain, vch[0])      # dD RAW
                vector.tensor_reduce(
                    out=valD[:, i * TPB:(i + 1) * TPB],
                    in_=dD[:].rearrange("p (t c) -> p t c", c=CB),
                    op=ALU.add,
                    axis=mybir.AxisListType.X,
                ).then_inc(dvedone, 1)

            def relus(i):
                if i >= 2:
                    vector.wait_ge(seldone, i - 1)
                for side in (0, 1):
                    for c in range(NCH):
                        if RELU_OWNER[(side, c)] == "v":
                            vector.wait_ge(
                                (ps_done if side == 0 else pd_done),
                                i * NCH + c + 1,
                            )
                            relu_vec(vector, i, side, c)

            for i in range(nbatch):
                relus(i)
                if i >= 1:
                    select(i - 1)
            select(nbatch - 1)
            vector.wait_ge(vchain, vch[0])
            vector.wait_ge(dvedone, nbatch)
            vector.tensor_tensor(
                out=out_sb[:], in0=valS[:], in1=valD[:], op=ALU.subtract,
            ).then_inc(fin, 1)

        @block.tensor
        def _(tensor):
            tensor.wait_ge(pre_load, 16 * NPRE)
            for i in range(nbatch):
                tensor.wait_ge(pload[i % 2], 32 * (i // 2 + 1))
                for c in range(NCH):
                    q = i * NCH + c
                    if q >= 2:
                        i2, c2 = divmod(q - 2, NCH)
                        sem2, n2 = r_cum(1, i2, c2)
                        tensor.wait_ge(sem2, n2)
                    tensor.matmul(
                        out=qD[q % 2][:],
                        lhsT=lhsT8[:],
                        rhs=pD_bufs[i % 2][:, c * CW:(c + 1) * CW],
                        start=True, stop=True,
                    ).then_inc(pd_done, 1)
                for c in range(NCH):
                    q = i * NCH + c
                    if q >= 2:
                        i2, c2 = divmod(q - 2, NCH)
                        sem2, n2 = r_cum(0, i2, c2)
                        tensor.wait_ge(sem2, n2)  # qS bank free
                    tensor.matmul(
                        out=qS[q % 2][:],
                        lhsT=lhsT8[:],
                        rhs=pS_bufs[i % 2][:, c * CW:(c + 1) * CW],
                        start=True, stop=True,
                    ).then_inc(ps_done, 1)
                if i == 0:
                    tensor.wait_ge(tbl_ready, 1)  # S_f16 ready
                if i >= 2:
                    tensor.wait_ge(dvedone, i - 1)
                for j in range(TPB):
                    if j % CHT == 0:
                        c = j // CHT
                        sem_s, n_s = r_cum(0, i, c)
                        tensor.wait_ge(sem_s, n_s)
                        sem_d, n_d = r_cum(1, i, c)
                        tensor.wait_ge(sem_d, n_d)
                    bs, bd = sched[i * TPB + j]
                    tensor.matmul(
                        out=psA[i % 2][:, j * CB:(j + 1) * CB],
                        lhsT=ohpS_bufs[i % 2][:, j * P:(j + 1) * P],
                        rhs=S_f16[:, bs * CB:(bs + 1) * CB],
                        start=True, stop=True,
                    )
                    mm = tensor.matmul(
                        out=psB[i % 2][:, j * CB:(j + 1) * CB],
                        lhsT=ohpD_bufs[i % 2][:, j * P:(j + 1) * P],
                        rhs=S_f16[:, bd * CB:(bd + 1) * CB],
                        start=True, stop=True,
                    )
                    if j == TPB - 1:
                        mm.then_inc(seldone, 1)

    return nc


def _prep(src_flat, dst_flat):
    E = src_flat.shape[0]
    assert E % NCORES == 0
    Ec = E // NCORES
    NG = NBLK * NBLK

    cores = []
    counts = np.zeros((NCORES, NG), np.int64)
    for i in range(NCORES):
        s = src_flat[i * Ec:(i + 1) * Ec].astype(np.int64)
        d = dst_flat[i * Ec:(i + 1) * Ec].astype(np.int64)
        g = (s >> 12) * NBLK + (d >> 12)
        order = np.argsort(g, kind="stable")
        cores.append((s[order], d[order], g[order], order + i * Ec))
        counts[i] = np.bincount(g, minlength=NG)

    gmax = counts.max(axis=0)
    tiles_per_group = (gmax + P - 1) // P
    n_tiles = int(tiles_per_group.sum())
    n_tiles_p = ((n_tiles + TPB - 1) // TPB) * TPB

    sched = []
    for gi in range(NG):
        sched.extend([(gi // NBLK, gi % NBLK)] * int(tiles_per_group[gi]))
    sched.extend([(0, 0)] * (n_tiles_p - n_tiles))

    slot_base = np.zeros(NG, np.int64)
    np.cumsum(tiles_per_group[:-1] * P, out=slot_base[1:])
    n_slots = n_tiles_p * P
    nbatch = n_tiles_p // TPB

    per_core = []
    for i in range(NCORES):
        s, d, g, orig = cores[i]
        cstart = np.zeros(NG, np.int64)
        np.cumsum(counts[i][:-1], out=cstart[1:])
        within = np.arange(Ec) - cstart[g]
        slot = slot_base[g] + within
        src_s = np.zeros(n_slots, np.int64)
        dst_s = np.zeros(n_slots, np.int64)
        src_s[slot] = s
        dst_s[slot] = d

        def p8(arr):
            pe = (arr & 127).astype(np.float32)
            ae = np.floor(pe / 8.0)
            be = pe - 8.0 * ae
            rows = [ae, ae, np.ones_like(ae), -(ae * ae), -2.0 * (be * be),
                    2.0 * be, -2.0 * (be * be) * 0.0 + np.ones_like(ae),
                    np.ones_like(ae)]
            out = np.stack(rows, axis=0).astype(NPF16)
            return out.reshape(8, nbatch, TPB * P).transpose(1, 0, 2).copy()

        def cmask(arr):
            # [nbatch, P(edge-in-tile), TPB*CB]: one-hot of c_e along CB
            c = ((arr >> 7) & 31).astype(np.int8).reshape(nbatch, TPB, P)
            oh = (c[:, :, :, None] == np.arange(CB, dtype=np.int8)).astype(
                NPBF
            )  # [nbatch, TPB, P(e), CB]
            return np.ascontiguousarray(
                oh.transpose(0, 2, 1, 3).reshape(nbatch, P, TPB * CB)
            )

        per_core.append(
            dict(
                p3_both=np.concatenate([p8(src_s), p8(dst_s)], axis=1),
                mask_both=np.concatenate([cmask(src_s), cmask(dst_s)], axis=2),
                slot=slot,
                orig=orig,
            )
        )
    return per_core, sched, n_tiles_p


def kernel(node_embeds, src_idx, dst_idx):
    node_embeds = np.asarray(node_embeds, dtype=np.float32)
    src_idx = np.asarray(src_idx)
    dst_idx = np.asarray(dst_idx)
    T, E = src_idx.shape
    n_nodes = node_embeds.shape[0]

    src_flat = src_idx.reshape(-1).astype(np.int64)
    dst_flat = dst_idx.reshape(-1).astype(np.int64)
    per_core, sched, n_tiles_p = _prep(src_flat, dst_flat)

    emb_pad = np.zeros((VPAD, D), np.float32)
    emb_pad[:n_nodes] = node_embeds

    iota = np.arange(P, dtype=np.float32)
    a = np.floor(iota / 8.0)
    b = iota - 8.0 * a
    one = np.ones(P, np.float32)
    lhsT8 = np.stack(
        [a, a, -(a * a), one, one, 2.0 * b, -2.0 * (b * b), one]
    ).astype(NPF16)

    nc = _build_nc(n_tiles_p, sched)
    in_maps = []
    for i in range(NCORES):
        pc = per_core[i]
        in_maps.append(
            {
                "embeds": emb_pad[i * NSH:(i + 1) * NSH],
                "p3_both": pc["p3_both"],
                "mask_in": pc["mask_both"],
                "lhsT8": lhsT8,
            }
        )
    res = run_bass_kernel_spmd(nc, in_maps, list(range(NCORES)))

    out_flat = np.zeros(T * E, np.float32)
    for i in range(NCORES):
        pc = per_core[i]
        yv = res.results[i]["y"]
        slot_vals = np.ascontiguousarray(yv.T).reshape(-1)
        out_flat[pc["orig"]] = slot_vals[pc["slot"]]
    return out_flat.reshape(T, E)
